# revision 6
# baseline (speedup 1.0000x reference)
"""DOTA mix E-step (vq_codebook) on 8 TRN2 NeuronCores.

out[b,k,m] = gamma_class[b,k] * softmax_m(-0.5*(log_det+maha) + log_pi)

Implicit-reference formulation: softmax over modes is shift-invariant, so
each class pins one reference mode r (the one with the largest constant
term) at logit 0 and the GEMM computes only the c-1 DIFFERENCE logits

  l''[b,j] = x2 . (W1_m - W1_r + dlc) + x . (W2_m - W2_r)

with W1 = -0.5/var, W2 = mu/var and the per-column constant dlc folded
uniformly into W1diff (legal since sum_d x^2 = 1 for unit-norm x). Then

  s = 1 + sum_j exp(l''), coef = gamma / s,
  out_m = coef * e_m (m != r), out_r = coef.

No-overflow guarantee: l'' <= max_d(W1diff) + ||W2diff|| (x2 lies on the
simplex, |x| = 1), checked on host to stay well under exp's f32 range.
This cuts packed GEMM columns from sum(c) to sum(c-1) (~438/core), which
fits ONE PSUM bank: no second column tile, no LDWEIGHTS stalls.

Classes are bucketed by width c-1, rounded to multiples of 8 by promoting
classes from the next-lower pool (one wasted -20000 column each) so all
cores run one SPMD program. Count-1 classes are exact on host (resp = 1).
Post-GEMM work (reduce / +1 / recip / coef / scale) runs once per 4-chunk
group to amortize per-instruction overhead; outputs stream back in f16.
"""

import sys

import numpy as np

sys.path.insert(0, "/opt/trn_rl_repo")

import concourse.bass as bass
import concourse.mybir as mybir
import concourse.tile as tile
from concourse import bacc, bass_utils

F32 = mybir.dt.float32
F16 = mybir.dt.float16

B, K, M, D = 4096, 1000, 8, 512
NCORES = 8
NB = B // 128             # 32 batch chunks of 128 rows
G = 4                     # chunks per post-processing group
EPS_REG = 1e-3
PAD_LOGIT = -20000.0      # exp -> 0 for promoted/dummy columns
MAX_LOGIT = 75.0          # overflow guard for exp in f32


def build_bass(buckets):
    """buckets: tuple of (width, n_classes_per_core) for widths 1..7."""
    nv = sum(w * n for w, n in buckets)       # packed diff columns per core
    kc = sum(n for _, n in buckets)           # packed classes per core
    assert nv <= 512, nv

    nc = bacc.Bacc("TRN2", debug=False, target_bir_lowering=False)
    xt = nc.dram_tensor("xt", (NB, 4, 128, 128), F16, kind="ExternalInput")
    x2t = nc.dram_tensor("x2t", (NB, 4, 128, 128), F16, kind="ExternalInput")
    w1 = nc.dram_tensor("w1", (4, 128, nv), F16, kind="ExternalInput")
    w2 = nc.dram_tensor("w2", (4, 128, nv), F16, kind="ExternalInput")
    gam = nc.dram_tensor("gam", (128, NB * kc), F32, kind="ExternalInput")
    out = nc.dram_tensor("out", (B, nv), F16, kind="ExternalOutput")
    cf = nc.dram_tensor("cf", (B, kc), F16, kind="ExternalOutput")
    warm = nc.dram_tensor("warm", (128, 128), F32, kind="ExternalOutput")

    xt_ap, x2t_ap, gam_ap, out_ap, cf_ap = (
        xt.ap(), x2t.ap(), gam.ap(), out.ap(), cf.ap())

    with tile.TileContext(nc) as tc:
        with (
            tc.tile_pool(name="wpool", bufs=1) as wpool,
            tc.tile_pool(name="xpool", bufs=4) as xpool,
            tc.tile_pool(name="ppool", bufs=4, space="PSUM") as ppool,
            tc.tile_pool(name="epool", bufs=2) as epool,
            tc.tile_pool(name="spool", bufs=2) as spool,
            tc.tile_pool(name="opool", bufs=2) as opool,
        ):
            # weight tiles; DMAs spread over the two free queues (scalar's
            # ring carries the per-chunk x loads) so they land fast
            w1t, w2t = [], []
            for r in range(4):
                t = wpool.tile([128, nv], F16, tag=f"w1_{r}")
                nc.sync.dma_start(t[:], w1.ap()[r])
                w1t.append(t)
            for r in range(4):
                t = wpool.tile([128, nv], F16, tag=f"w2_{r}")
                nc.gpsimd.dma_start(t[:], w2.ap()[r])
                w2t.append(t)
            gall = wpool.tile([128, NB * kc], F32, tag="gam")
            nc.gpsimd.dma_start(gall[:], gam_ap[:, :])

            # HAM warmup: dummy matmuls while DMAs land, so the real GEMM
            # starts at 2.4 GHz instead of 1.2
            wz = wpool.tile([128, 128], F16, tag="warmz")
            nc.vector.memset(wz[:], 0.0)
            wps = ppool.tile([128, 512], F32, tag="wps")
            for i in range(40):
                nc.tensor.matmul(wps[:, 0:128], lhsT=wz[:], rhs=wz[:],
                                 start=True, stop=True)
            wsb = wpool.tile([128, 128], F32, tag="warmsb")
            nc.vector.tensor_copy(wsb[:], wps[:, 0:128])
            nc.sync.dma_start(warm.ap()[:, :], wsb[:])

            for bc in range(NB):
                q, g = bc % G, bc // G
                if bc % 2 == 0:
                    xb2 = xpool.tile([128, 1024], F16, tag="xb")
                    nc.scalar.dma_start(
                        xb2[:].rearrange("p (c r j) -> p c r j", c=2, r=4),
                        xt_ap[bc:bc + 2].rearrange("c r p j -> p c r j"))
                    x2b2 = xpool.tile([128, 1024], F16, tag="x2b")
                    nc.scalar.dma_start(
                        x2b2[:].rearrange("p (c r j) -> p c r j", c=2, r=4),
                        x2t_ap[bc:bc + 2].rearrange("c r p j -> p c r j"))
                xoff = (bc % 2) * 512

                ps = ppool.tile([128, 512], F32, tag="ps")
                for r in range(4):
                    nc.tensor.matmul(
                        ps[:, 0:nv],
                        lhsT=x2b2[:, xoff + r * 128:xoff + (r + 1) * 128],
                        rhs=w1t[r][:], start=(r == 0), stop=False)
                for r in range(4):
                    nc.tensor.matmul(
                        ps[:, 0:nv],
                        lhsT=xb2[:, xoff + r * 128:xoff + (r + 1) * 128],
                        rhs=w2t[r][:], start=False, stop=(r == 3))

                if q == 0:
                    ew = epool.tile([128, G * nv], F32, tag="ew")
                nc.scalar.activation(ew[:, q * nv:(q + 1) * nv], ps[:, 0:nv],
                                     mybir.ActivationFunctionType.Exp)
                if q != G - 1:
                    continue

                ew3 = ew[:].rearrange("p (c v) -> p c v", v=nv)
                ssum = spool.tile([128, G * kc], F32, tag="ssum")
                ss3 = ssum[:].rearrange("p (c k) -> p c k", c=G)
                off = koff = 0
                for w, n in buckets:
                    nc.vector.reduce_sum(
                        ss3[:, :, koff:koff + n],
                        ew3[:, :, off:off + n * w].rearrange(
                            "p c (k m) -> p c k m", m=w),
                        axis=mybir.AxisListType.X)
                    off += n * w
                    koff += n
                nc.vector.tensor_scalar_add(ssum[:], ssum[:], 1.0)
                rec = spool.tile([128, G * kc], F32, tag="rec")
                nc.vector.reciprocal_approx_fast(rec[:], ssum[:])
                coef = spool.tile([128, G * kc], F32, tag="coef")
                nc.vector.tensor_mul(
                    coef[:], rec[:], gall[:, g * G * kc:(g + 1) * G * kc])
                cfo = spool.tile([128, G * kc], F16, tag="cfo")
                nc.scalar.activation(cfo[:], coef[:],
                                     mybir.ActivationFunctionType.Copy)

                o4 = opool.tile([128, G * nv], F16, tag="o4")
                o3 = o4[:].rearrange("p (c v) -> p c v", v=nv)
                c3 = coef[:].rearrange("p (c k) -> p c k", c=G)
                off = koff = 0
                for w, n in buckets:
                    e4 = ew3[:, :, off:off + n * w].rearrange(
                        "p c (k m) -> p c k m", m=w)
                    ob = o3[:, :, off:off + n * w].rearrange(
                        "p c (k m) -> p c k m", m=w)
                    cb = c3[:, :, koff:koff + n].rearrange(
                        "p c (k one) -> p c k one", one=1)
                    e4b, cbb = bass.broadcast_tensor_aps(e4, cb)
                    nc.gpsimd.tensor_tensor(ob, e4b, cbb,
                                            op=mybir.AluOpType.mult)
                    off += n * w
                    koff += n
                rows = slice(g * G * 128, (g + 1) * G * 128)
                nc.sync.dma_start(
                    out_ap[rows, :].rearrange("(c p) j -> p c j", p=128),
                    o4[:].rearrange("p (c j) -> p c j", c=G))
                nc.gpsimd.dma_start(
                    cf_ap[rows, :].rearrange("(c p) k -> p c k", p=128),
                    cfo[:].rearrange("p (c k) -> p c k", c=G))

    nc.compile()
    return nc


def _layout(mask):
    """Bucket classes by diff-width w = count-1 (count-1 classes are host
    handled); round each bucket to a multiple of NCORES by promoting classes
    from the next-lower pool (cost: 1 wasted column each); remaining gaps in
    the w=1 bucket get dummies (-1)."""
    counts = np.asarray(mask, bool).sum(-1).astype(int)     # (K,)
    pools = {w: list(np.where(counts == w + 1)[0]) for w in range(1, M)}
    entries = []
    for w in range(M - 1, 0, -1):
        ids = pools[w]
        pools[w] = []
        pad = (-len(ids)) % NCORES
        if pad and w > 1 and len(pools[w - 1]) >= pad:
            ids += pools[w - 1][:pad]
            pools[w - 1] = pools[w - 1][pad:]
        elif pad:
            ids += [-1] * pad
        if ids:
            entries.append((w, ids))
    entries.sort()
    per_core = [[] for _ in range(NCORES)]
    buckets = []
    for w, ids in entries:
        n = len(ids) // NCORES
        buckets.append((w, n))
        for c in range(NCORES):
            per_core[c].append((w, ids[c * n:(c + 1) * n]))
    ones = np.where(counts == 1)[0]
    return tuple(buckets), per_core, ones


def prep_inputs(x, gamma_class, mu_pad, var_pad, pi_pad, mask):
    x = np.asarray(x, np.float32)
    gamma_class = np.asarray(gamma_class, np.float32)
    mask = np.asarray(mask, bool)
    counts = mask.sum(-1).astype(int)

    var = np.clip(np.asarray(var_pad, np.float64) + EPS_REG, 1e-8, None)
    inv = 1.0 / var
    W1 = -0.5 * inv                                    # (K, M, D)
    W2 = np.asarray(mu_pad, np.float64) * inv
    logdet = np.log(var).sum(-1)
    muinvmu = (np.asarray(mu_pad, np.float64) * W2).sum(-1)
    logpi = np.where(mask, np.log(np.asarray(pi_pad, np.float64) + 1e-10),
                     -np.inf)
    lc = -0.5 * logdet - 0.5 * muinvmu + logpi          # (K, M)

    lc_valid = np.where(mask, lc, -np.inf)
    ref = np.argmax(lc_valid, axis=1)                   # (K,)

    def class_bound(k, r):
        c = counts[k]
        ms = [m for m in range(c) if m != r]
        if not ms:
            return -np.inf
        dW1 = W1[k, ms] - W1[k, r] + (lc[k, ms] - lc[k, r])[:, None]
        dW2 = W2[k, ms] - W2[k, r]
        return (dW1.max(-1) + np.sqrt((dW2 ** 2).sum(-1))).max()

    # overflow guard: exp stays finite in f32; re-pick ref if needed
    for k in np.where(counts >= 2)[0]:
        if class_bound(k, ref[k]) > MAX_LOGIT:
            cand = [(class_bound(k, r), r) for r in range(counts[k])]
            bd, r = min(cand)
            if bd > MAX_LOGIT:
                raise ValueError(f"class {k}: logit bound {bd:.1f} > "
                                 f"{MAX_LOGIT}; scheme unsafe")
            ref[k] = r

    buckets, per_core, ones = _layout(mask)
    nv = sum(w * n for w, n in buckets)
    kc = sum(n for _, n in buckets)

    x16 = x.astype(np.float16)
    xtb = np.ascontiguousarray(
        x16.reshape(NB, 128, 4, 128).transpose(0, 2, 3, 1))
    x2tb = np.ascontiguousarray(
        (x16.astype(np.float32) ** 2).astype(np.float16)
        .reshape(NB, 128, 4, 128).transpose(0, 2, 3, 1))

    in_maps, metas = [], []
    for cidx in range(NCORES):
        # unused/promoted cols: every element PAD_LOGIT/D so the folded
        # constant sums to PAD_LOGIT (Sx2=1) -> exp ~ 1e-17, never scattered
        w1c = np.full((nv, D), PAD_LOGIT / D, np.float64)
        w2c = np.zeros((nv, D), np.float64)
        gcols = np.zeros((B, kc), np.float32)
        col_cls = np.full(nv, -1, np.int64)
        col_mode = np.zeros(nv, np.int64)
        kcls = np.full(kc, -1, np.int64)
        kref = np.zeros(kc, np.int64)
        off = koff = 0
        for w, ids in per_core[cidx]:
            for k in ids:
                if k >= 0:
                    c, r = counts[k], ref[k]
                    ms = [m for m in range(c) if m != r]
                    nm = len(ms)
                    w1c[off:off + nm] = (W1[k, ms] - W1[k, r]
                                         + (lc[k, ms] - lc[k, r])[:, None])
                    w2c[off:off + nm] = W2[k, ms] - W2[k, r]
                    # promoted classes: unused cols stay at exp->0
                    col_cls[off:off + nm] = k
                    col_mode[off:off + nm] = ms
                    kcls[koff] = k
                    kref[koff] = r
                    gcols[:, koff] = gamma_class[:, k]
                off += w
                koff += 1
        # the PAD_LOGIT/D init makes unused cols sum to PAD_LOGIT via Sx2=1
        in_maps.append({
            "xt": xtb,
            "x2t": x2tb,
            "w1": np.ascontiguousarray(
                w1c.T.astype(np.float16).reshape(4, 128, nv)),
            "w2": np.ascontiguousarray(
                w2c.T.astype(np.float16).reshape(4, 128, nv)),
            "gam": np.ascontiguousarray(
                gcols.reshape(NB, 128, kc).transpose(1, 0, 2)
                .reshape(128, NB * kc)),
        })
        metas.append((col_cls, col_mode, kcls, kref))
    return in_maps, buckets, metas, ones


def scatter_core(out, packed, cfp, meta):
    """Scatter one core's packed (B, nv) diffs + (B, kc) coefs into out."""
    col_cls, col_mode, kcls, kref = meta
    real = col_cls >= 0
    out[:, col_cls[real], col_mode[real]] = packed[:, real]
    realk = kcls >= 0
    out[:, kcls[realk], kref[realk]] = cfp[:, realk]


_NC_CACHE = {}


def _get_nc(buckets):
    if buckets not in _NC_CACHE:
        _NC_CACHE[buckets] = build_bass(buckets)
    return _NC_CACHE[buckets]


def kernel(x, gamma_class, mu_pad, var_pad, pi_pad, mask, _trace=False):
    in_maps, buckets, metas, ones = prep_inputs(
        x, gamma_class, mu_pad, var_pad, pi_pad, mask)
    gamma_class = np.asarray(gamma_class, np.float32)
    out = np.zeros((B, K, M), np.float32)
    if len(ones):
        out[:, ones, 0] = gamma_class[:, ones]
    if not buckets:
        return out
    nc = _get_nc(buckets)
    res = bass_utils.run_bass_kernel_spmd(
        nc, in_maps, core_ids=list(range(NCORES)), trace=_trace)
    for cidx in range(NCORES):
        scatter_core(out, res.results[cidx]["out"].astype(np.float32),
                     res.results[cidx]["cf"].astype(np.float32), metas[cidx])
    if _trace:
        kernel.last_results = res
    return out


# revision 9
# speedup vs baseline: 1.2373x; 1.2373x over previous
"""DOTA mix E-step (vq_codebook) on 8 TRN2 NeuronCores.

out[b,k,m] = gamma_class[b,k] * softmax_m(-0.5*(log_det+maha) + log_pi)

Implicit-reference formulation: softmax over modes is shift-invariant, so
each class pins one reference mode r (the one with the largest constant
term) at logit 0 and the GEMM computes only the c-1 DIFFERENCE logits

  l''[b,j] = x2 . (W1_m - W1_r + dlc) + x . (W2_m - W2_r)

with W1 = -0.5/var, W2 = mu/var and the per-column constant dlc folded
uniformly into W1diff (legal since sum_d x^2 = 1 for unit-norm x). Then

  s = 1 + sum_j exp(l''), coef = gamma / s,
  out_m = coef * e_m (m != r), out_r = coef.

No-overflow guarantee: l'' <= max_d(W1diff) + ||W2diff|| (x2 lies on the
simplex, |x| = 1), checked on host to stay well under exp's f32 range.
This cuts packed GEMM columns from sum(c) to sum(c-1) (~438/core), which
fits ONE PSUM bank: no second column tile, no LDWEIGHTS stalls.

Classes are bucketed by width c-1, rounded to multiples of 8 by promoting
classes from the next-lower pool (one wasted -20000 column each) so all
cores run one SPMD program. Count-1 classes are exact on host (resp = 1).
Post-GEMM work (reduce / +1 / recip / coef / scale) runs once per 4-chunk
group to amortize per-instruction overhead; outputs stream back in f16.
"""

import sys

import numpy as np

sys.path.insert(0, "/opt/trn_rl_repo")

import concourse.bass as bass
import concourse.mybir as mybir
import concourse.tile as tile
from concourse import bacc, bass_utils

F32 = mybir.dt.float32
F16 = mybir.dt.float16

B, K, M, D = 4096, 1000, 8, 512
NCORES = 8
NB = B // 128             # 32 batch chunks of 128 rows
G = 4                     # chunks per post-processing group
EPS_REG = 1e-3
PAD_LOGIT = -20000.0      # exp -> 0 for promoted/dummy columns
MAX_LOGIT = 75.0          # overflow guard for exp in f32


def build_bass(buckets):
    """buckets: tuple of (width, n_classes_per_core) for widths 1..7."""
    nv = sum(w * n for w, n in buckets)       # packed diff columns per core
    kc = sum(n for _, n in buckets)           # packed classes per core
    assert nv <= 512, nv

    nc = bacc.Bacc("TRN2", debug=False, target_bir_lowering=False)
    xt = nc.dram_tensor("xt", (NB, 4, 128, 128), F16, kind="ExternalInput")
    x2t = nc.dram_tensor("x2t", (NB, 4, 128, 128), F16, kind="ExternalInput")
    w1 = nc.dram_tensor("w1", (4, 128, nv), F16, kind="ExternalInput")
    w2 = nc.dram_tensor("w2", (4, 128, nv), F16, kind="ExternalInput")
    gam = nc.dram_tensor("gam", (128, NB * kc), F32, kind="ExternalInput")
    out = nc.dram_tensor("out", (B, nv), F16, kind="ExternalOutput")
    cf = nc.dram_tensor("cf", (B, kc), F16, kind="ExternalOutput")
    warm = nc.dram_tensor("warm", (128, 128), F32, kind="ExternalOutput")

    xt_ap, x2t_ap, gam_ap, out_ap, cf_ap = (
        xt.ap(), x2t.ap(), gam.ap(), out.ap(), cf.ap())

    with tile.TileContext(nc) as tc:
        with (
            tc.tile_pool(name="wpool", bufs=1) as wpool,
            tc.tile_pool(name="xpool", bufs=6) as xpool,
            tc.tile_pool(name="ppool", bufs=4, space="PSUM") as ppool,
            tc.tile_pool(name="epool", bufs=3) as epool,
            tc.tile_pool(name="spool", bufs=2) as spool,
            tc.tile_pool(name="opool", bufs=3) as opool,
        ):
            # weight tiles; DMAs spread over the two free queues (scalar's
            # ring carries the per-chunk x loads) so they land fast
            w1t, w2t = [], []
            for r in range(4):
                t = wpool.tile([128, nv], F16, tag=f"w1_{r}")
                nc.sync.dma_start(t[:], w1.ap()[r])
                w1t.append(t)
            for r in range(4):
                t = wpool.tile([128, nv], F16, tag=f"w2_{r}")
                nc.gpsimd.dma_start(t[:], w2.ap()[r])
                w2t.append(t)
            gall = wpool.tile([128, NB * kc], F32, tag="gam")
            nc.gpsimd.dma_start(gall[:], gam_ap[:, :])

            # HAM warmup: dummy matmuls while DMAs land, so the real GEMM
            # starts at 2.4 GHz instead of 1.2. One accumulation group so
            # consecutive MMs pipeline at N cycles instead of paying a full
            # fill+drain each.
            wz = wpool.tile([128, 128], F16, tag="warmz")
            nc.gpsimd.memset(wz[:], 0.0)
            wps = ppool.tile([128, 512], F32, tag="wps")
            for i in range(40):
                nc.tensor.matmul(wps[:, 0:128], lhsT=wz[:], rhs=wz[:],
                                 start=(i == 0), stop=(i == 39))
            wsb = wpool.tile([128, 128], F32, tag="warmsb")
            nc.vector.tensor_copy(wsb[:], wps[:, 0:128])
            nc.sync.dma_start(warm.ap()[:, :], wsb[:])

            # x/x2 pair loads, prefetched PF pairs ahead of use
            PF = 2
            xtiles = {}

            def load_pair(p):
                if p >= NB // 2:
                    return
                xb2 = xpool.tile([128, 1024], F16, tag="xb")
                nc.scalar.dma_start(
                    xb2[:].rearrange("p (c r j) -> p c r j", c=2, r=4),
                    xt_ap[2 * p:2 * p + 2].rearrange("c r p j -> p c r j"))
                x2b2 = xpool.tile([128, 1024], F16, tag="x2b")
                nc.scalar.dma_start(
                    x2b2[:].rearrange("p (c r j) -> p c r j", c=2, r=4),
                    x2t_ap[2 * p:2 * p + 2].rearrange("c r p j -> p c r j"))
                xtiles[p] = (xb2, x2b2)

            for p in range(PF + 1):
                load_pair(p)

            for bc in range(NB):
                q, g = bc % G, bc // G
                if bc % 2 == 0:
                    load_pair(bc // 2 + PF + 1)
                    xb2, x2b2 = xtiles.pop(bc // 2)
                xoff = (bc % 2) * 512

                ps = ppool.tile([128, 512], F32, tag="ps")
                for r in range(4):
                    nc.tensor.matmul(
                        ps[:, 0:nv],
                        lhsT=x2b2[:, xoff + r * 128:xoff + (r + 1) * 128],
                        rhs=w1t[r][:], start=(r == 0), stop=False)
                for r in range(4):
                    nc.tensor.matmul(
                        ps[:, 0:nv],
                        lhsT=xb2[:, xoff + r * 128:xoff + (r + 1) * 128],
                        rhs=w2t[r][:], start=False, stop=(r == 3))

                if q == 0:
                    ew = epool.tile([128, G * nv], F32, tag="ew")
                nc.scalar.activation(ew[:, q * nv:(q + 1) * nv], ps[:, 0:nv],
                                     mybir.ActivationFunctionType.Exp)
                if q != G - 1:
                    continue

                ew3 = ew[:].rearrange("p (c v) -> p c v", v=nv)
                ssum = spool.tile([128, G * kc], F32, tag="ssum")
                ss3 = ssum[:].rearrange("p (c k) -> p c k", c=G)
                off = koff = 0
                for w, n in buckets:
                    nc.vector.reduce_sum(
                        ss3[:, :, koff:koff + n],
                        ew3[:, :, off:off + n * w].rearrange(
                            "p c (k m) -> p c k m", m=w),
                        axis=mybir.AxisListType.X)
                    off += n * w
                    koff += n
                nc.vector.tensor_scalar_add(ssum[:], ssum[:], 1.0)
                rec = spool.tile([128, G * kc], F32, tag="rec")
                nc.vector.reciprocal_approx_fast(rec[:], ssum[:])
                coef = spool.tile([128, G * kc], F32, tag="coef")
                nc.vector.tensor_mul(
                    coef[:], rec[:], gall[:, g * G * kc:(g + 1) * G * kc])
                cfo = spool.tile([128, G * kc], F16, tag="cfo")
                nc.scalar.activation(cfo[:], coef[:],
                                     mybir.ActivationFunctionType.Copy)

                o4 = opool.tile([128, G * nv], F16, tag="o4")
                o3 = o4[:].rearrange("p (c v) -> p c v", v=nv)
                c3 = coef[:].rearrange("p (c k) -> p c k", c=G)
                off = koff = 0
                for w, n in buckets:
                    e4 = ew3[:, :, off:off + n * w].rearrange(
                        "p c (k m) -> p c k m", m=w)
                    ob = o3[:, :, off:off + n * w].rearrange(
                        "p c (k m) -> p c k m", m=w)
                    cb = c3[:, :, koff:koff + n].rearrange(
                        "p c (k one) -> p c k one", one=1)
                    e4b, cbb = bass.broadcast_tensor_aps(e4, cb)
                    # small buckets on DVE (low fixed cost), large on GPSIMD
                    eng = nc.vector if w <= 3 else nc.gpsimd
                    eng.tensor_tensor(ob, e4b, cbb, op=mybir.AluOpType.mult)
                    off += n * w
                    koff += n
                rows = slice(g * G * 128, (g + 1) * G * 128)
                nc.sync.dma_start(
                    out_ap[rows, :].rearrange("(c p) j -> p c j", p=128),
                    o4[:].rearrange("p (c j) -> p c j", c=G))
                nc.gpsimd.dma_start(
                    cf_ap[rows, :].rearrange("(c p) k -> p c k", p=128),
                    cfo[:].rearrange("p (c k) -> p c k", c=G))

    nc.compile()
    return nc


def _layout(mask):
    """Bucket classes by diff-width w = count-1 (count-1 classes are host
    handled); round each bucket to a multiple of NCORES by promoting classes
    from the next-lower pool (cost: 1 wasted column each); remaining gaps in
    the w=1 bucket get dummies (-1)."""
    counts = np.asarray(mask, bool).sum(-1).astype(int)     # (K,)
    pools = {w: list(np.where(counts == w + 1)[0]) for w in range(1, M)}
    entries = []
    for w in range(M - 1, 0, -1):
        ids = pools[w]
        pools[w] = []
        pad = (-len(ids)) % NCORES
        if pad and w > 1 and len(pools[w - 1]) >= pad:
            ids += pools[w - 1][:pad]
            pools[w - 1] = pools[w - 1][pad:]
        elif pad:
            ids += [-1] * pad
        if ids:
            entries.append((w, ids))
    entries.sort()
    per_core = [[] for _ in range(NCORES)]
    buckets = []
    for w, ids in entries:
        n = len(ids) // NCORES
        buckets.append((w, n))
        for c in range(NCORES):
            per_core[c].append((w, ids[c * n:(c + 1) * n]))
    ones = np.where(counts == 1)[0]
    return tuple(buckets), per_core, ones


def prep_inputs(x, gamma_class, mu_pad, var_pad, pi_pad, mask):
    x = np.asarray(x, np.float32)
    gamma_class = np.asarray(gamma_class, np.float32)
    mask = np.asarray(mask, bool)
    counts = mask.sum(-1).astype(int)

    var = np.clip(np.asarray(var_pad, np.float64) + EPS_REG, 1e-8, None)
    inv = 1.0 / var
    W1 = -0.5 * inv                                    # (K, M, D)
    W2 = np.asarray(mu_pad, np.float64) * inv
    logdet = np.log(var).sum(-1)
    muinvmu = (np.asarray(mu_pad, np.float64) * W2).sum(-1)
    logpi = np.where(mask, np.log(np.asarray(pi_pad, np.float64) + 1e-10),
                     -np.inf)
    lc = -0.5 * logdet - 0.5 * muinvmu + logpi          # (K, M)

    lc_valid = np.where(mask, lc, -np.inf)
    ref = np.argmax(lc_valid, axis=1)                   # (K,)

    def class_bound(k, r):
        c = counts[k]
        ms = [m for m in range(c) if m != r]
        if not ms:
            return -np.inf
        dW1 = W1[k, ms] - W1[k, r] + (lc[k, ms] - lc[k, r])[:, None]
        dW2 = W2[k, ms] - W2[k, r]
        return (dW1.max(-1) + np.sqrt((dW2 ** 2).sum(-1))).max()

    # overflow guard: exp stays finite in f32; re-pick ref if needed
    for k in np.where(counts >= 2)[0]:
        if class_bound(k, ref[k]) > MAX_LOGIT:
            cand = [(class_bound(k, r), r) for r in range(counts[k])]
            bd, r = min(cand)
            if bd > MAX_LOGIT:
                raise ValueError(f"class {k}: logit bound {bd:.1f} > "
                                 f"{MAX_LOGIT}; scheme unsafe")
            ref[k] = r

    buckets, per_core, ones = _layout(mask)
    nv = sum(w * n for w, n in buckets)
    kc = sum(n for _, n in buckets)

    x16 = x.astype(np.float16)
    xtb = np.ascontiguousarray(
        x16.reshape(NB, 128, 4, 128).transpose(0, 2, 3, 1))
    x2tb = np.ascontiguousarray(
        (x16.astype(np.float32) ** 2).astype(np.float16)
        .reshape(NB, 128, 4, 128).transpose(0, 2, 3, 1))

    in_maps, metas = [], []
    for cidx in range(NCORES):
        # unused/promoted cols: every element PAD_LOGIT/D so the folded
        # constant sums to PAD_LOGIT (Sx2=1) -> exp ~ 1e-17, never scattered
        w1c = np.full((nv, D), PAD_LOGIT / D, np.float64)
        w2c = np.zeros((nv, D), np.float64)
        gcols = np.zeros((B, kc), np.float32)
        col_cls = np.full(nv, -1, np.int64)
        col_mode = np.zeros(nv, np.int64)
        kcls = np.full(kc, -1, np.int64)
        kref = np.zeros(kc, np.int64)
        off = koff = 0
        for w, ids in per_core[cidx]:
            for k in ids:
                if k >= 0:
                    c, r = counts[k], ref[k]
                    ms = [m for m in range(c) if m != r]
                    nm = len(ms)
                    w1c[off:off + nm] = (W1[k, ms] - W1[k, r]
                                         + (lc[k, ms] - lc[k, r])[:, None])
                    w2c[off:off + nm] = W2[k, ms] - W2[k, r]
                    # promoted classes: unused cols stay at exp->0
                    col_cls[off:off + nm] = k
                    col_mode[off:off + nm] = ms
                    kcls[koff] = k
                    kref[koff] = r
                    gcols[:, koff] = gamma_class[:, k]
                off += w
                koff += 1
        # the PAD_LOGIT/D init makes unused cols sum to PAD_LOGIT via Sx2=1
        in_maps.append({
            "xt": xtb,
            "x2t": x2tb,
            "w1": np.ascontiguousarray(
                w1c.T.astype(np.float16).reshape(4, 128, nv)),
            "w2": np.ascontiguousarray(
                w2c.T.astype(np.float16).reshape(4, 128, nv)),
            "gam": np.ascontiguousarray(
                gcols.reshape(NB, 128, kc).transpose(1, 0, 2)
                .reshape(128, NB * kc)),
        })
        metas.append((col_cls, col_mode, kcls, kref))
    return in_maps, buckets, metas, ones


def scatter_core(out, packed, cfp, meta):
    """Scatter one core's packed (B, nv) diffs + (B, kc) coefs into out."""
    col_cls, col_mode, kcls, kref = meta
    real = col_cls >= 0
    out[:, col_cls[real], col_mode[real]] = packed[:, real]
    realk = kcls >= 0
    out[:, kcls[realk], kref[realk]] = cfp[:, realk]


_NC_CACHE = {}


def _get_nc(buckets):
    if buckets not in _NC_CACHE:
        _NC_CACHE[buckets] = build_bass(buckets)
    return _NC_CACHE[buckets]


def kernel(x, gamma_class, mu_pad, var_pad, pi_pad, mask, _trace=False):
    in_maps, buckets, metas, ones = prep_inputs(
        x, gamma_class, mu_pad, var_pad, pi_pad, mask)
    gamma_class = np.asarray(gamma_class, np.float32)
    out = np.zeros((B, K, M), np.float32)
    if len(ones):
        out[:, ones, 0] = gamma_class[:, ones]
    if not buckets:
        return out
    nc = _get_nc(buckets)
    res = bass_utils.run_bass_kernel_spmd(
        nc, in_maps, core_ids=list(range(NCORES)), trace=_trace)
    for cidx in range(NCORES):
        scatter_core(out, res.results[cidx]["out"].astype(np.float32),
                     res.results[cidx]["cf"].astype(np.float32), metas[cidx])
    if _trace:
        kernel.last_results = res
    return out
